# revision 27
# baseline (speedup 1.0000x reference)
"""Trainium2 Bass kernel for GroupedQueryAttention (inverted sliding-window mask + sink).

Full inputs in, full output out. Internally head-sharded across 8 NeuronCores:
core c handles q heads {2c, 2c+1} and kv head c//2, computes its partial
(x @ Wqkv_slice -> RoPE -> scores -> masked softmax w/ sink -> AV -> @ Wo_slice),
host sums the 8 partial outputs (the all-reduce).

v3 design notes:
- bf16 datapath end to end (inputs, SBUF slabs, output partials).
- head-dim rows of q/k are permuted on the host so each RoPE rotation pair
  sits on adjacent partitions; the partner operand is then a single DVE
  stream_shuffle (swap-adjacent within quadrants) -- no cross-partition DMA.
  Rows 64:128 of the permuted layout don't rotate and are left untouched.
- xT is preloaded whole into SBUF; projection runs in two l-block pairs
  consuming xT chunks in DMA arrival order.
- inverted-band mask: score/exp/AV/den matmuls are column-trimmed to the
  active range of each (k-tile, q-block) tile; the one 128-col boundary
  stripe is zeroed with a multiplicative bf16 mask on DVE.
- y partials are written as [128, 2048] slabs (16 output DMAs).
"""

import os
import sys
from contextlib import ExitStack

sys.path.insert(0, "/opt/trn_rl_repo")

# jax must see the axon/neuron platform; a stray JAX_PLATFORMS=cpu would hide it.
if os.environ.get("JAX_PLATFORMS", "") == "cpu":
    os.environ["JAX_PLATFORMS"] = ""

import numpy as np

import concourse.bass as bass
import concourse.tile as tile
from concourse import bacc, mybir

F32 = mybir.dt.float32
BF16 = mybir.dt.bfloat16

N_CORES = 8
L = 2048
D = 2048
HD = 128
WINDOW = 1024
ROPE_BASE = 1024.0
SM_SCALE = 1.0 / float(np.sqrt(HD))

MASK_VAL = -1.0e6
QB = 512          # q block (free dim of score tiles)
NQB = L // QB     # 4
NKT = L // HD     # 16 k tiles of 128
NDK = D // HD     # 16 contraction chunks for projections
NLB = L // QB     # 4 l-blocks for projection

LOWER_D0S = (0, -128, -256, -384)
UPPER_D0S = (640, 768, 896, 1024)
MASK_IDX = {d: i for i, d in enumerate(LOWER_D0S + UPPER_D0S)}

# swap-adjacent stream_shuffle mask (within each 32-partition quadrant)
SWAP_ADJ = [j ^ 1 for j in range(32)]

# qb=1's first tiles only need l-blocks 1-2, hiding the tail of phase A
QB_ORDER = (1, 2, 3, 0)


def _classify(kt: int, qb: int):
    """masked band is 0 <= q-k <= WINDOW-1 (those entries are dropped).

    Returns (kind, d0, c0, c1, m0, m1): active column range [c0, c1) and
    boundary mask-stripe [m0, m1) for the (k-tile, q-block) tile.
    """
    d0 = QB * qb - HD * kt
    if 128 <= d0 <= 512:
        return "skip", d0, 0, 0, 0, 0
    if d0 <= -512 or d0 >= 1152:
        return "full", d0, 0, QB, 0, 0
    if d0 <= 0:
        w = min(QB, 128 - d0)
        return "lower", d0, 0, w, w - 128, w
    off = max(0, 1024 - d0)
    return "upper", d0, off, QB, off, off + 128


def _build_program():
    nc = bacc.Bacc("TRN2", target_bir_lowering=False, debug=False,
                   num_devices=N_CORES)

    # xT and wslc are pre-swizzled on the host into SBUF layout
    # ([partition, chunk, free]) so input DMAs are few and pattern-identical.
    xT_d = nc.dram_tensor("xT", [128, NDK, L], BF16, kind="ExternalInput").ap()
    wslc_d = nc.dram_tensor("wslc", [128, NDK, 4 * HD], BF16,
                            kind="ExternalInput").ap()
    wo_d = nc.dram_tensor("wo", [2 * HD, D], BF16, kind="ExternalInput").ap()
    snk_d = nc.dram_tensor("snk", [1, 2], F32, kind="ExternalInput").ap()
    cosd_d = nc.dram_tensor("cosd", [64, L], BF16, kind="ExternalInput").ap()
    sind_d = nc.dram_tensor("sind", [64, L], BF16, kind="ExternalInput").ap()
    y_d = nc.dram_tensor("y", [L, D], BF16, kind="ExternalOutput").ap()

    with tile.TileContext(nc) as tc, ExitStack() as stk:
        persist = stk.enter_context(tc.tile_pool(name="persist", bufs=1))

        # ---- persistent SBUF tensors ----
        xT_sb = persist.tile([128, NDK, L], BF16, tag="xT")
        wslc_sb = persist.tile([128, NDK, 4 * HD], BF16, tag="wslc")
        wo_sb = persist.tile([128, 2, D], BF16, tag="wo")
        qT = [persist.tile([128, L], BF16, tag=f"qT{h}", name=f"qT{h}") for h in range(2)]
        kT = persist.tile([128, L], BF16, tag="kT")
        v_sb = persist.tile([128, NKT, HD], BF16, tag="v")
        oT = [persist.tile([128, L], BF16, tag=f"oT{h}", name=f"oT{h}") for h in range(2)]
        cosd_sb = persist.tile([64, L], BF16, tag="cosd")
        sind_sb = persist.tile([64, L], BF16, tag="sind")
        # additive boundary stripes: stripes[:,0,:] masks where c >= k'
        # (lower-edge tiles), stripes[:,1,:] masks where c < k' (upper-edge)
        stripes = persist.tile([128, 2, 128], BF16, tag="stripes")
        ident = persist.tile([128, 128], BF16, tag="ident")
        ones_f32 = persist.tile([128, 1], F32, tag="onesf")
        ones_sb = persist.tile([128, 1], BF16, tag="ones")
        snk_sb = persist.tile([1, 2], F32, tag="snk")
        exps_sb = persist.tile([1, 2], F32, tag="exps")

        # ---- input DMAs ----
        # sync queue: wslc + xT interleaved in consumption order.  lb0's
        # x-tiles arrive in fine k-group pieces so the first projection
        # matmuls start ~2.5us in; later l-blocks use coarser pieces.
        nc.sync.dma_start(wslc_sb[:, 0:2, :], wslc_d[:, 0:2, :])
        nc.sync.dma_start(xT_sb[:, 0:1, 0:QB], xT_d[:, 0:1, 0:QB])
        nc.sync.dma_start(xT_sb[:, 1:2, 0:QB], xT_d[:, 1:2, 0:QB])
        nc.sync.dma_start(wslc_sb[:, 2:4, :], wslc_d[:, 2:4, :])
        nc.sync.dma_start(xT_sb[:, 2:4, 0:QB], xT_d[:, 2:4, 0:QB])
        nc.sync.dma_start(wslc_sb[:, 4:6, :], wslc_d[:, 4:6, :])
        nc.sync.dma_start(xT_sb[:, 4:6, 0:QB], xT_d[:, 4:6, 0:QB])
        nc.sync.dma_start(wslc_sb[:, 6:8, :], wslc_d[:, 6:8, :])
        nc.sync.dma_start(xT_sb[:, 6:8, 0:QB], xT_d[:, 6:8, 0:QB])
        nc.sync.dma_start(wslc_sb[:, 8:12, :], wslc_d[:, 8:12, :])
        nc.sync.dma_start(xT_sb[:, 8:12, 0:QB], xT_d[:, 8:12, 0:QB])
        nc.sync.dma_start(wslc_sb[:, 12:16, :], wslc_d[:, 12:16, :])
        nc.sync.dma_start(xT_sb[:, 12:16, 0:QB], xT_d[:, 12:16, 0:QB])
        for lb in range(1, NLB):
            ls = slice(lb * QB, (lb + 1) * QB)
            nc.sync.dma_start(xT_sb[:, 0:8, ls], xT_d[:, 0:8, ls])
            nc.sync.dma_start(xT_sb[:, 8:16, ls], xT_d[:, 8:16, ls])
        # gpsimd (software-DGE) queue: rope tables, sink, Wo
        nc.gpsimd.dma_start(cosd_sb[:], cosd_d[:])
        nc.gpsimd.dma_start(sind_sb[:], sind_d[:])
        nc.gpsimd.dma_start(snk_sb[:], snk_d[:])
        for h in range(2):
            nc.gpsimd.dma_start(wo_sb[:, h, :], wo_d[h * 128:(h + 1) * 128, :])

        # ---- constants ----
        nc.gpsimd.memset(ones_f32[:], 1.0)
        nc.scalar.copy(ones_sb[:], ones_f32[:])
        # identity for PE transposes
        nc.gpsimd.memset(ident[:], 0.0)
        nc.gpsimd.affine_select(
            out=ident[:], in_=ident[:], compare_op=mybir.AluOpType.not_equal,
            fill=1.0, base=0, channel_multiplier=1, pattern=[[-1, 128]])
        # additive boundary stripes (0 kept, -1e6 dropped); every partial
        # tile's masked wedge is one of these two patterns at offset m0
        for i in range(2):
            m = stripes[:, i, :]
            nc.gpsimd.memset(m, 0.0)
            if i == 0:
                # lower edge: keep where c < k'
                nc.gpsimd.affine_select(
                    out=m, in_=m, compare_op=mybir.AluOpType.is_ge,
                    fill=MASK_VAL, base=-1, channel_multiplier=1,
                    pattern=[[-1, 128]])
            else:
                # upper edge: keep where c >= k'
                nc.gpsimd.affine_select(
                    out=m, in_=m, compare_op=mybir.AluOpType.is_ge,
                    fill=MASK_VAL, base=0, channel_multiplier=-1,
                    pattern=[[1, 128]])
        # exp of the two sink logits
        nc.scalar.activation(exps_sb[:], snk_sb[:], mybir.ActivationFunctionType.Exp)

        # ================= Phase A: QKV projection (transposed) =================
        # pT[c*128+r, l] = sum_d wslc[d, c*128+r] * x[l, d];  cols c = q0,q1,k,v
        vt_pool = stk.enter_context(tc.tile_pool(name="vtp", bufs=2))
        rope_pool = stk.enter_context(tc.tile_pool(name="rope", bufs=2))
        sbB = stk.enter_context(tc.tile_pool(name="sbB", bufs=8))
        sbY = stk.enter_context(tc.tile_pool(name="sbY", bufs=2))
        with tc.tile_pool(name="psA", bufs=8, space="PSUM") as psA:
            vt_q = []

            def _emit_v_transposes(item):
                dlb, dvt = item
                for j in range(QB // 128):
                    kt = dlb * (QB // 128) + j
                    pt = psA.tile([128, 128], BF16, tag="proj")
                    nc.tensor.transpose(pt[:], dvt[:, j * 128:(j + 1) * 128],
                                        ident[:])
                    nc.scalar.copy(v_sb[:, kt, :], pt[:])

            for lb in range(NLB):
                ls = slice(lb * QB, (lb + 1) * QB)
                ps = [psA.tile([128, QB], F32, tag="proj", name=f"ps{lb}c{c}")
                      for c in range(4)]
                for k in range(NDK):
                    if k == 5 and vt_q:
                        _emit_v_transposes(vt_q.pop(0))
                    for c in range(4):
                        nc.tensor.matmul(
                            ps[c][:],
                            wslc_sb[:, k, c * 128:(c + 1) * 128],
                            xT_sb[:, k, ls],
                            start=(k == 0), stop=(k == NDK - 1))
                # drain psums: q0,q1 on ACT; k,v on DVE
                nc.scalar.copy(qT[0][:, ls], ps[0][:])
                nc.scalar.copy(qT[1][:, ls], ps[1][:])
                nc.vector.tensor_copy(kT[:, ls], ps[2][:])
                vT_lb = vt_pool.tile([128, QB], BF16, tag="vt")
                nc.vector.tensor_copy(vT_lb[:], ps[3][:])

                # ---- RoPE on rows 0:64 of this l-block (in place) ----
                for t in (qT[0], qT[1], kT):
                    u = rope_pool.tile([64, QB], BF16, tag="u")
                    tmp = rope_pool.tile([64, QB], BF16, tag="tmp")
                    nc.vector.stream_shuffle(u[:], t[0:64, ls], SWAP_ADJ)
                    nc.vector.tensor_mul(tmp[:], t[0:64, ls], cosd_sb[:, ls])
                    nc.vector.tensor_mul(u[:], u[:], sind_sb[:, ls])
                    nc.vector.tensor_add(t[0:64, ls], tmp[:], u[:])

                # ---- v: transpose to natural (k, d) tiles, deferred one
                # l-block so the PE never waits on the vT drain copy ----
                vt_q.append((lb, vT_lb))
            while vt_q:
                _emit_v_transposes(vt_q.pop(0))

        # ============ Phase B+C: attention + output projection ============
        # Full tiles are processed in pairs sharing a two-bank PSUM tile so
        # one ACT exp instruction covers both (halves ACT instruction count);
        # partial (boundary) tiles follow, column-trimmed to the active range.
        with tc.tile_pool(name="psS", bufs=2, space="PSUM") as psS, \
             tc.tile_pool(name="psO", bufs=1, space="PSUM") as psO, \
             tc.tile_pool(name="psD", bufs=1, space="PSUM") as psD, \
             tc.tile_pool(name="psY", bufs=2, space="PSUM") as psY:
            for qb in QB_ORDER:
                qs = slice(qb * QB, (qb + 1) * QB)
                for h in range(2):
                    acts = [(kt, *_classify(kt, qb)) for kt in range(NKT)]
                    fulls = [a for a in acts if a[1] == "full"]
                    parts = [a for a in acts if a[1] in ("lower", "upper")]
                    n_act = len(fulls) + len(parts)
                    # pair partials with equal active ranges when possible
                    parts.sort(key=lambda a: (a[3], a[4]))
                    fpairs = [fulls[p:p + 2] for p in range(0, len(fulls), 2)]
                    ppairs = [parts[p:p + 2] for p in range(0, len(parts), 2)]
                    # full pairs first (the first opens all 512 psum cols),
                    # then the column-trimmed partial pairs
                    order = fpairs + ppairs
                    psum_o = psO.tile([128, QB], F32, tag="o")
                    psum_den = psD.tile([1, QB], F32, tag="den")
                    i = 0
                    last_group = (qb == QB_ORDER[-1] and h == 1)
                    deferred_av = []

                    def av_den(kt, e_ap, cs, first, last):
                        nc.tensor.matmul(
                            psum_den[:, cs], ones_sb[:], e_ap,
                            start=first, stop=last)
                        nc.tensor.matmul(
                            psum_o[:, cs], v_sb[:, kt, :], e_ap,
                            start=first, stop=last)

                    for pair in order:
                        ps2 = psS.tile([128, 2, QB], F32, tag="s")
                        e2 = sbB.tile([128, 2, QB], BF16, tag="e")
                        for j, (kt, cls, d0, c0, c1, m0, m1) in enumerate(pair):
                            cs = slice(c0, c1)
                            nc.tensor.matmul(
                                ps2[:, j, cs],
                                kT[:, kt * 128:(kt + 1) * 128],
                                qT[h][:, qb * QB + c0:qb * QB + c1],
                                start=True, stop=(cls == "full"))
                            if cls != "full":
                                # add the -1e6 boundary stripe in PSUM (PE,
                                # N=128) so exp sees pre-masked scores
                                nc.tensor.matmul(
                                    ps2[:, j, m0:m1], ident[:],
                                    stripes[:, 0 if cls == "lower" else 1, :],
                                    start=False, stop=True)
                        r0 = (pair[0][3], pair[0][4])
                        r1 = (pair[-1][3], pair[-1][4])
                        if r0 == r1 and len(pair) == 2:
                            # same active range: one exp over both halves
                            nc.scalar.activation(
                                e2[:, :, r0[0]:r0[1]], ps2[:, :, r0[0]:r0[1]],
                                mybir.ActivationFunctionType.Exp,
                                scale=SM_SCALE)
                        else:
                            for j, (kt, cls, d0, c0, c1, m0, m1) in enumerate(pair):
                                nc.scalar.activation(
                                    e2[:, j, c0:c1], ps2[:, j, c0:c1],
                                    mybir.ActivationFunctionType.Exp,
                                    scale=SM_SCALE)
                        if last_group:
                            for j, (kt, cls, d0, c0, c1, m0, m1) in enumerate(pair):
                                nc.tensor.matmul(
                                    psum_den[:, c0:c1], ones_sb[:],
                                    e2[:, j, c0:c1],
                                    start=(i + j == 0), stop=(i + j == n_act - 1))
                            deferred_av.append((pair, e2))
                            i += len(pair)
                        else:
                            for j, (kt, cls, d0, c0, c1, m0, m1) in enumerate(pair):
                                av_den(kt, e2[:, j, c0:c1], slice(c0, c1),
                                       i == 0, i == n_act - 1)
                                i += 1
                    for idx, (pair, e2) in enumerate(deferred_av):
                        for j, (kt, cls, d0, c0, c1, m0, m1) in enumerate(pair):
                            first = (idx == 0 and j == 0)
                            last = (idx == len(deferred_av) - 1
                                    and j == len(pair) - 1)
                            nc.tensor.matmul(
                                psum_o[:, c0:c1], v_sb[:, kt, :],
                                e2[:, j, c0:c1],
                                start=first, stop=last)
                    # free psum_o with a plain copy; normalize in SBUF once
                    # the reciprocal-broadcast chain lands (off PE's path)
                    o_un = sbB.tile([128, QB], F32, tag="oun")
                    nc.vector.tensor_copy(o_un[:], psum_o[:])
                    den_sb = sbB.tile([1, QB], F32, tag="densb")
                    nc.scalar.activation(
                        den_sb[:], psum_den[:],
                        mybir.ActivationFunctionType.Identity,
                        bias=exps_sb[0:1, h:h + 1])
                    r_sb = sbB.tile([1, QB], F32, tag="rsb")
                    nc.vector.reciprocal(r_sb[:], den_sb[:])
                    rb = sbB.tile([128, QB], F32, tag="rb")
                    nc.gpsimd.partition_broadcast(rb[:], r_sb[:])
                    nc.vector.tensor_mul(oT[h][:, qs], o_un[:], rb[:])

                # ---- Wo for this q block ----
                for j in range(QB // 128):
                    qt = qb * (QB // 128) + j
                    qts = slice(qt * 128, (qt + 1) * 128)
                    y_slab = sbY.tile([128, D], BF16, tag="ysb")
                    for nb in range(D // QB):
                        ns = slice(nb * QB, (nb + 1) * QB)
                        psum_y = psY.tile([128, QB], F32, tag="y")
                        for h in range(2):
                            nc.tensor.matmul(
                                psum_y[:],
                                oT[h][:, qts],
                                wo_sb[:, h, ns],
                                start=(h == 0), stop=(h == 1))
                        if (qt + nb) % 2 == 0:
                            nc.vector.tensor_copy(y_slab[:, ns], psum_y[:])
                        else:
                            nc.scalar.copy(y_slab[:, ns], psum_y[:])
                    nc.sync.dma_start(y_d[qts, :], y_slab[:])

    nc.compile()
    return nc


# head-dim permutation: rotation pair (i, i+64) -> partitions (2i, 2i+1);
# non-rotating dims 32:64 -> 64:96, 96:128 stay.
PERM = np.zeros(HD, dtype=np.int64)
for _i in range(32):
    PERM[2 * _i] = _i
    PERM[2 * _i + 1] = 64 + _i
for _j in range(32):
    PERM[64 + _j] = 32 + _j
    PERM[96 + _j] = 96 + _j


def _rope_tables():
    import ml_dtypes
    freqs = (1.0 / ROPE_BASE) ** np.linspace(0.0, 1.0, num=HD // 4,
                                             dtype=np.float32)
    theta = freqs[:, None].astype(np.float32) * np.arange(L, dtype=np.float32)[None, :]
    cos32 = np.cos(theta).astype(np.float32)
    sin32 = np.sin(theta).astype(np.float32)
    cos64 = np.empty((64, L), dtype=np.float32)
    sin64 = np.empty((64, L), dtype=np.float32)
    cos64[0::2] = cos32
    cos64[1::2] = cos32
    sin64[0::2] = sin32
    sin64[1::2] = -sin32
    return (cos64.astype(ml_dtypes.bfloat16), sin64.astype(ml_dtypes.bfloat16))


def _make_in_maps(x, Wqkv, Wo, s):
    import ml_dtypes
    bf16 = ml_dtypes.bfloat16
    x = np.asarray(x, dtype=np.float32)
    Wqkv = np.asarray(Wqkv, dtype=np.float32)
    Wo = np.asarray(Wo, dtype=np.float32)
    s = np.asarray(s, dtype=np.float32)
    # swizzle into SBUF layout [partition, chunk, l]:
    # xT_sw[r, k, l] = x[l, 128k + r]
    xT = np.ascontiguousarray(
        x.reshape(L, NDK, 128).transpose(2, 1, 0)).astype(bf16)
    cosd, sind = _rope_tables()
    in_maps = []
    for c in range(N_CORES):
        g = c // 2
        qcols = [Wqkv[:, (2 * c + hh) * HD:(2 * c + hh + 1) * HD][:, PERM]
                 for hh in range(2)]
        kcols = Wqkv[:, 16 * HD + g * HD:16 * HD + (g + 1) * HD][:, PERM]
        vcols = Wqkv[:, 20 * HD + g * HD:20 * HD + (g + 1) * HD]
        wslc = np.concatenate(qcols + [kcols, vcols], axis=1)
        # wslc_sw[r, k, cc] = wslc[128k + r, cc]
        wslc_sw = np.ascontiguousarray(
            wslc.reshape(NDK, 128, 4 * HD).transpose(1, 0, 2)).astype(bf16)
        in_maps.append({
            "xT": xT,
            "wslc": wslc_sw,
            "wo": np.ascontiguousarray(Wo[(2 * c) * HD:(2 * c + 2) * HD, :]).astype(bf16),
            "snk": np.ascontiguousarray(s[:, 2 * c:2 * c + 2]),
            "cosd": cosd,
            "sind": sind,
        })
    return in_maps


_CACHE = {}


def _get_exec():
    """Build the program once and return a cached jitted 8-core executor."""
    if "exec" in _CACHE:
        return _CACHE["exec"]

    import jax
    from jax.sharding import Mesh, PartitionSpec
    from jax.experimental.shard_map import shard_map
    from concourse.bass2jax import (_bass_exec_p, install_neuronx_cc_hook,
                                    partition_id_tensor)

    nc = _build_program()
    install_neuronx_cc_hook()

    partition_name = (nc.partition_id_tensor.name
                      if nc.partition_id_tensor else None)
    in_names, out_names, out_avals = [], [], []
    for alloc in nc.m.functions[0].allocations:
        if not isinstance(alloc, mybir.MemoryLocationSet):
            continue
        name = alloc.memorylocations[0].name
        if alloc.kind == "ExternalInput":
            if name != partition_name:
                in_names.append(name)
        elif alloc.kind == "ExternalOutput":
            out_names.append(name)
            out_avals.append(jax.core.ShapedArray(
                tuple(alloc.tensor_shape), mybir.dt.np(alloc.dtype)))
    n_params = len(in_names)
    all_names = in_names + out_names
    if partition_name is not None:
        all_names = all_names + [partition_name]

    def _body(*args):
        operands = list(args)
        if partition_name is not None:
            operands.append(partition_id_tensor())
        outs = _bass_exec_p.bind(
            *operands,
            out_avals=tuple(out_avals),
            in_names=tuple(all_names),
            out_names=tuple(out_names),
            lowering_input_output_aliases=(),
            sim_require_finite=True,
            sim_require_nnan=True,
            nc=nc,
        )
        return tuple(outs)

    devices = jax.devices()[:N_CORES]
    mesh = Mesh(np.asarray(devices), ("core",))
    n_outs = len(out_names)
    sharded = jax.jit(
        shard_map(_body, mesh=mesh,
                  in_specs=(PartitionSpec("core"),) * (n_params + n_outs),
                  out_specs=(PartitionSpec("core"),) * n_outs,
                  check_rep=False),
        keep_unused=True)

    state = {
        "sharded": sharded, "in_names": in_names, "out_names": out_names,
        "out_avals": out_avals, "mesh": mesh, "n_params": n_params,
    }
    _CACHE["exec"] = state
    return state


def _run_cores(in_maps):
    ex = _get_exec()
    concat_in = [
        np.concatenate([np.asarray(m[name]) for m in in_maps], axis=0)
        for name in ex["in_names"]
    ]
    concat_zeros = [
        np.zeros((N_CORES * a.shape[0],) + tuple(a.shape[1:]), a.dtype)
        for a in ex["out_avals"]
    ]
    outs = ex["sharded"](*concat_in, *concat_zeros)
    name_to_i = {n: i for i, n in enumerate(ex["out_names"])}
    yi = name_to_i["y"]
    y_all = np.asarray(outs[yi]).reshape(N_CORES, L, D)
    return y_all


def kernel(x, Wqkv, Wo, s):
    in_maps = _make_in_maps(x, Wqkv, Wo, s)
    y_all = _run_cores(in_maps)
    out = y_all.astype(np.float32).sum(axis=0)
    return out.reshape(1, L, D).astype(np.float32)


# revision 36
# speedup vs baseline: 1.3062x; 1.3062x over previous
"""Trainium2 Bass kernel for GroupedQueryAttention (inverted sliding-window mask + sink).

Full inputs in, full output out. Internally head-sharded across 8 NeuronCores:
core c handles q heads {2c, 2c+1} and kv head c//2, computes its partial
(x @ Wqkv_slice -> RoPE -> scores -> masked softmax w/ sink -> AV -> @ Wo_slice),
host sums the 8 partial outputs (the all-reduce).

v3 design notes:
- bf16 datapath end to end (inputs, SBUF slabs, output partials).
- head-dim rows of q/k are permuted on the host so each RoPE rotation pair
  sits on adjacent partitions; the partner operand is then a single DVE
  stream_shuffle (swap-adjacent within quadrants) -- no cross-partition DMA.
  Rows 64:128 of the permuted layout don't rotate and are left untouched.
- xT is preloaded whole into SBUF; projection runs in two l-block pairs
  consuming xT chunks in DMA arrival order.
- inverted-band mask: score/exp/AV/den matmuls are column-trimmed to the
  active range of each (k-tile, q-block) tile; the one 128-col boundary
  stripe is zeroed with a multiplicative bf16 mask on DVE.
- y partials are written as [128, 2048] slabs (16 output DMAs).
"""

import os
import sys
from contextlib import ExitStack

sys.path.insert(0, "/opt/trn_rl_repo")

# jax must see the axon/neuron platform; a stray JAX_PLATFORMS=cpu would hide it.
if os.environ.get("JAX_PLATFORMS", "") == "cpu":
    os.environ["JAX_PLATFORMS"] = ""

import numpy as np

import concourse.bass as bass
import concourse.tile as tile
from concourse import bacc, mybir

F32 = mybir.dt.float32
BF16 = mybir.dt.bfloat16

N_CORES = 8
L = 2048
D = 2048
HD = 128
WINDOW = 1024
ROPE_BASE = 1024.0
SM_SCALE = 1.0 / float(np.sqrt(HD))

MASK_VAL = -1.0e6
QB = 512          # q block (free dim of score tiles)
NQB = L // QB     # 4
NKT = L // HD     # 16 k tiles of 128
NDK = D // HD     # 16 contraction chunks for projections
NLB = L // QB     # 4 l-blocks for projection

LOWER_D0S = (0, -128, -256, -384)
UPPER_D0S = (640, 768, 896, 1024)
MASK_IDX = {d: i for i, d in enumerate(LOWER_D0S + UPPER_D0S)}

# swap-adjacent stream_shuffle mask (within each 32-partition quadrant)
SWAP_ADJ = [j ^ 1 for j in range(32)]

# qb=1's first tiles only need l-blocks 1-2, hiding the tail of phase A
QB_ORDER = (1, 2, 3, 0)


def _classify(kt: int, qb: int):
    """masked band is 0 <= q-k <= WINDOW-1 (those entries are dropped).

    Returns (kind, d0, c0, c1, m0, m1): active column range [c0, c1) and
    boundary mask-stripe [m0, m1) for the (k-tile, q-block) tile.
    """
    d0 = QB * qb - HD * kt
    if 128 <= d0 <= 512:
        return "skip", d0, 0, 0, 0, 0
    if d0 <= -512 or d0 >= 1152:
        return "full", d0, 0, QB, 0, 0
    if d0 <= 0:
        w = min(QB, 128 - d0)
        return "lower", d0, 0, w, w - 128, w
    off = max(0, 1024 - d0)
    return "upper", d0, off, QB, off, off + 128


def _build_program():
    nc = bacc.Bacc("TRN2", target_bir_lowering=False, debug=False,
                   num_devices=N_CORES)

    # xT and wslc are pre-swizzled on the host into SBUF layout
    # ([partition, chunk, free]) so input DMAs are few and pattern-identical.
    xT_d = nc.dram_tensor("xT", [128, NDK, L], BF16, kind="ExternalInput").ap()
    wslc_d = nc.dram_tensor("wslc", [128, NDK, 4 * HD], BF16,
                            kind="ExternalInput").ap()
    wo_d = nc.dram_tensor("wo", [2 * HD, D], BF16, kind="ExternalInput").ap()
    snk_d = nc.dram_tensor("snk", [1, 2], F32, kind="ExternalInput").ap()
    cosd_d = nc.dram_tensor("cosd", [64, L], BF16, kind="ExternalInput").ap()
    sind_d = nc.dram_tensor("sind", [64, L], BF16, kind="ExternalInput").ap()
    y_d = nc.dram_tensor("y", [L, D], BF16, kind="ExternalOutput").ap()

    with tile.TileContext(nc) as tc, ExitStack() as stk:
        persist = stk.enter_context(tc.tile_pool(name="persist", bufs=1))

        # ---- persistent SBUF tensors ----
        xT_sb = persist.tile([128, NDK, L], BF16, tag="xT")
        wslc_sb = persist.tile([128, NDK, 4 * HD], BF16, tag="wslc")
        wo_sb = persist.tile([128, 2, D], BF16, tag="wo")
        qT = [persist.tile([128, L], BF16, tag=f"qT{h}", name=f"qT{h}") for h in range(2)]
        kT = persist.tile([128, L], BF16, tag="kT")
        v_sb = persist.tile([128, NKT, HD], BF16, tag="v")
        oT = [persist.tile([128, L], BF16, tag=f"oT{h}", name=f"oT{h}") for h in range(2)]
        cosd_sb = persist.tile([64, L], BF16, tag="cosd")
        sind_sb = persist.tile([64, L], BF16, tag="sind")
        # additive boundary stripes: stripes[:,0,:] masks where c >= k'
        # (lower-edge tiles), stripes[:,1,:] masks where c < k' (upper-edge)
        stripes = persist.tile([128, 2, 128], BF16, tag="stripes")
        ident = persist.tile([128, 128], BF16, tag="ident")
        ones_f32 = persist.tile([128, 1], F32, tag="onesf")
        ones_sb = persist.tile([128, 1], BF16, tag="ones")
        snk_sb = persist.tile([1, 2], F32, tag="snk")
        exps_sb = persist.tile([1, 2], F32, tag="exps")

        # ---- input DMAs ----
        # sync queue: wslc + xT interleaved in consumption order.  lb0's
        # x-tiles arrive in fine k-group pieces so the first projection
        # matmuls start ~2.5us in; later l-blocks use coarser pieces.
        nc.sync.dma_start(wslc_sb[:, 0:2, :], wslc_d[:, 0:2, :])
        nc.sync.dma_start(xT_sb[:, 0:1, 0:QB], xT_d[:, 0:1, 0:QB])
        nc.sync.dma_start(xT_sb[:, 1:2, 0:QB], xT_d[:, 1:2, 0:QB])
        nc.sync.dma_start(wslc_sb[:, 2:4, :], wslc_d[:, 2:4, :])
        nc.sync.dma_start(xT_sb[:, 2:4, 0:QB], xT_d[:, 2:4, 0:QB])
        nc.sync.dma_start(wslc_sb[:, 4:6, :], wslc_d[:, 4:6, :])
        nc.sync.dma_start(xT_sb[:, 4:6, 0:QB], xT_d[:, 4:6, 0:QB])
        nc.sync.dma_start(wslc_sb[:, 6:8, :], wslc_d[:, 6:8, :])
        nc.sync.dma_start(xT_sb[:, 6:8, 0:QB], xT_d[:, 6:8, 0:QB])
        nc.sync.dma_start(wslc_sb[:, 8:12, :], wslc_d[:, 8:12, :])
        nc.sync.dma_start(xT_sb[:, 8:12, 0:QB], xT_d[:, 8:12, 0:QB])
        nc.sync.dma_start(wslc_sb[:, 12:16, :], wslc_d[:, 12:16, :])
        nc.sync.dma_start(xT_sb[:, 12:16, 0:QB], xT_d[:, 12:16, 0:QB])
        for lb in range(1, NLB):
            ls = slice(lb * QB, (lb + 1) * QB)
            nc.sync.dma_start(xT_sb[:, 0:8, ls], xT_d[:, 0:8, ls])
            nc.sync.dma_start(xT_sb[:, 8:16, ls], xT_d[:, 8:16, ls])
        # gpsimd (software-DGE) queue: rope tables, sink, Wo
        nc.gpsimd.dma_start(cosd_sb[:], cosd_d[:])
        nc.gpsimd.dma_start(sind_sb[:], sind_d[:])
        nc.gpsimd.dma_start(snk_sb[:], snk_d[:])
        for h in range(2):
            nc.gpsimd.dma_start(wo_sb[:, h, :], wo_d[h * 128:(h + 1) * 128, :])

        # ---- constants ----
        nc.gpsimd.memset(ones_f32[:], 1.0)
        nc.scalar.copy(ones_sb[:], ones_f32[:])
        # identity for PE transposes
        nc.gpsimd.memset(ident[:], 0.0)
        nc.gpsimd.affine_select(
            out=ident[:], in_=ident[:], compare_op=mybir.AluOpType.not_equal,
            fill=1.0, base=0, channel_multiplier=1, pattern=[[-1, 128]])
        # additive boundary stripes (0 kept, -1e6 dropped); every partial
        # tile's masked wedge is one of these two patterns at offset m0
        for i in range(2):
            m = stripes[:, i, :]
            nc.gpsimd.memset(m, 0.0)
            if i == 0:
                # lower edge: keep where c < k'
                nc.gpsimd.affine_select(
                    out=m, in_=m, compare_op=mybir.AluOpType.is_ge,
                    fill=MASK_VAL, base=-1, channel_multiplier=1,
                    pattern=[[-1, 128]])
            else:
                # upper edge: keep where c >= k'
                nc.gpsimd.affine_select(
                    out=m, in_=m, compare_op=mybir.AluOpType.is_ge,
                    fill=MASK_VAL, base=0, channel_multiplier=-1,
                    pattern=[[1, 128]])
        # exp of the two sink logits
        nc.scalar.activation(exps_sb[:], snk_sb[:], mybir.ActivationFunctionType.Exp)

        # ================= Phase A: QKV projection (transposed) =================
        # pT[c*128+r, l] = sum_d wslc[d, c*128+r] * x[l, d];  cols c = q0,q1,k,v
        vt_pool = stk.enter_context(tc.tile_pool(name="vtp", bufs=2))
        rope_pool = stk.enter_context(tc.tile_pool(name="rope", bufs=2))
        sbB = stk.enter_context(tc.tile_pool(name="sbB", bufs=4))
        sbE = stk.enter_context(tc.tile_pool(name="sbE", bufs=8))
        sbY = stk.enter_context(tc.tile_pool(name="sbY", bufs=3))
        with tc.tile_pool(name="psA", bufs=8, space="PSUM") as psA:
            vt_q = []

            def _emit_v_transposes(item):
                dlb, dvt = item
                for j in range(QB // 128):
                    kt = dlb * (QB // 128) + j
                    pt = psA.tile([128, 128], BF16, tag="proj")
                    nc.tensor.transpose(pt[:], dvt[:, j * 128:(j + 1) * 128],
                                        ident[:])
                    nc.scalar.copy(v_sb[:, kt, :], pt[:])

            for lb in range(NLB):
                ls = slice(lb * QB, (lb + 1) * QB)
                ps = [psA.tile([128, QB], F32, tag="proj", name=f"ps{lb}c{c}")
                      for c in range(4)]
                for k in range(NDK):
                    if k == 5 and vt_q:
                        _emit_v_transposes(vt_q.pop(0))
                    for c in range(4):
                        nc.tensor.matmul(
                            ps[c][:],
                            wslc_sb[:, k, c * 128:(c + 1) * 128],
                            xT_sb[:, k, ls],
                            start=(k == 0), stop=(k == NDK - 1))
                # drain psums: q0,q1 on ACT; k,v on DVE
                nc.scalar.copy(qT[0][:, ls], ps[0][:])
                nc.scalar.copy(qT[1][:, ls], ps[1][:])
                nc.vector.tensor_copy(kT[:, ls], ps[2][:])
                vT_lb = vt_pool.tile([128, QB], BF16, tag="vt")
                nc.vector.tensor_copy(vT_lb[:], ps[3][:])

                # ---- RoPE on rows 0:64 of this l-block (in place) ----
                for t in (qT[0], qT[1], kT):
                    u = rope_pool.tile([64, QB], BF16, tag="u")
                    tmp = rope_pool.tile([64, QB], BF16, tag="tmp")
                    nc.vector.stream_shuffle(u[:], t[0:64, ls], SWAP_ADJ)
                    nc.vector.tensor_mul(tmp[:], t[0:64, ls], cosd_sb[:, ls])
                    nc.vector.tensor_mul(u[:], u[:], sind_sb[:, ls])
                    nc.vector.tensor_add(t[0:64, ls], tmp[:], u[:])

                # ---- v: transpose to natural (k, d) tiles, deferred one
                # l-block so the PE never waits on the vT drain copy ----
                vt_q.append((lb, vT_lb))
            while vt_q:
                _emit_v_transposes(vt_q.pop(0))

        # ============ Phase B+C: attention + output projection ============
        # Full tiles are processed in pairs sharing a two-bank PSUM tile so
        # one ACT exp instruction covers both (halves ACT instruction count);
        # partial (boundary) tiles follow, column-trimmed to the active range.
        with tc.tile_pool(name="psS", bufs=2, space="PSUM") as psS, \
             tc.tile_pool(name="psO", bufs=1, space="PSUM") as psO, \
             tc.tile_pool(name="psD", bufs=1, space="PSUM") as psD, \
             tc.tile_pool(name="psY", bufs=2, space="PSUM") as psY:
            for qb in QB_ORDER:
                qs = slice(qb * QB, (qb + 1) * QB)
                for h in range(2):
                    acts = [(kt, *_classify(kt, qb)) for kt in range(NKT)]
                    fulls = [a for a in acts if a[1] == "full"]
                    parts = [a for a in acts if a[1] in ("lower", "upper")]
                    n_act = len(fulls) + len(parts)
                    # pair partials with equal active ranges when possible
                    from collections import defaultdict as _dd
                    byrange = _dd(list)
                    for a in parts:
                        byrange[(a[3], a[4])].append(a)
                    ppairs, odd = [], []
                    for key in sorted(byrange):
                        grp = byrange[key]
                        while len(grp) >= 2:
                            ppairs.append([grp.pop(0), grp.pop(0)])
                        if grp:
                            odd.append(grp.pop())
                    while len(odd) >= 2:
                        ppairs.append([odd.pop(0), odd.pop(0)])
                    if odd:
                        ppairs.append([odd.pop()])
                    fpairs = [fulls[p:p + 2] for p in range(0, len(fulls), 2)]
                    # full pairs first (the first opens all 512 psum cols),
                    # then the column-trimmed partial pairs
                    order = fpairs + ppairs
                    psum_o = psO.tile([128, QB], F32, tag="o")
                    psum_den = psD.tile([1, QB], F32, tag="den")
                    i = 0
                    last_group = (qb == QB_ORDER[-1] and h == 1)
                    deferred_av = []
                    # den instruction count: one per equal-range pair, one
                    # per half otherwise
                    n_den = sum(1 if (len(p) == 2 and (p[0][3], p[0][4]) ==
                                      (p[1][3], p[1][4])) else len(p)
                                for p in fpairs + ppairs)
                    den_i = 0

                    def av_den(kt, e_ap, cs, first, last):
                        nc.tensor.matmul(
                            psum_den[:, cs], ones_sb[:], e_ap,
                            start=first, stop=last)
                        nc.tensor.matmul(
                            psum_o[:, cs], v_sb[:, kt, :], e_ap,
                            start=first, stop=last)

                    for pair in order:
                        ps2 = psS.tile([128, 2, QB], F32, tag="s")
                        e2 = sbE.tile([128, 2, QB], BF16, tag="e")
                        for j, (kt, cls, d0, c0, c1, m0, m1) in enumerate(pair):
                            cs = slice(c0, c1)
                            nc.tensor.matmul(
                                ps2[:, j, cs],
                                kT[:, kt * 128:(kt + 1) * 128],
                                qT[h][:, qb * QB + c0:qb * QB + c1],
                                start=True, stop=(cls == "full"))
                            if cls != "full":
                                # add the -1e6 boundary stripe in PSUM (PE,
                                # N=128) so exp sees pre-masked scores
                                nc.tensor.matmul(
                                    ps2[:, j, m0:m1], ident[:],
                                    stripes[:, 0 if cls == "lower" else 1, :],
                                    start=False, stop=True)
                        r0 = (pair[0][3], pair[0][4])
                        r1 = (pair[-1][3], pair[-1][4])
                        if r0 == r1 and len(pair) == 2:
                            # same active range: one exp over both halves
                            nc.scalar.activation(
                                e2[:, :, r0[0]:r0[1]], ps2[:, :, r0[0]:r0[1]],
                                mybir.ActivationFunctionType.Exp,
                                scale=SM_SCALE)
                        else:
                            for j, (kt, cls, d0, c0, c1, m0, m1) in enumerate(pair):
                                nc.scalar.activation(
                                    e2[:, j, c0:c1], ps2[:, j, c0:c1],
                                    mybir.ActivationFunctionType.Exp,
                                    scale=SM_SCALE)
                        eqrange = (len(pair) == 2 and
                                   (pair[0][3], pair[0][4]) ==
                                   (pair[1][3], pair[1][4]))
                        if eqrange:
                            # fold the pair's den into one matmul via a bf16
                            # DVE add of the two e halves
                            c0, c1 = pair[0][3], pair[0][4]
                            esum = sbB.tile([128, QB], BF16, tag="esum")
                            nc.vector.tensor_add(
                                esum[:, c0:c1], e2[:, 0, c0:c1],
                                e2[:, 1, c0:c1])
                            nc.tensor.matmul(
                                psum_den[:, c0:c1], ones_sb[:],
                                esum[:, c0:c1],
                                start=(den_i == 0), stop=(den_i == n_den - 1))
                            den_i += 1
                        else:
                            for j, (kt, cls, d0, c0, c1, m0, m1) in enumerate(pair):
                                nc.tensor.matmul(
                                    psum_den[:, c0:c1], ones_sb[:],
                                    e2[:, j, c0:c1],
                                    start=(den_i == 0), stop=(den_i == n_den - 1))
                                den_i += 1
                        if last_group:
                            deferred_av.append((pair, e2))
                            i += len(pair)
                        else:
                            for j, (kt, cls, d0, c0, c1, m0, m1) in enumerate(pair):
                                nc.tensor.matmul(
                                    psum_o[:, c0:c1], v_sb[:, kt, :],
                                    e2[:, j, c0:c1],
                                    start=(i == 0), stop=(i == n_act - 1))
                                i += 1
                    for idx, (pair, e2) in enumerate(deferred_av):
                        for j, (kt, cls, d0, c0, c1, m0, m1) in enumerate(pair):
                            first = (idx == 0 and j == 0)
                            last = (idx == len(deferred_av) - 1
                                    and j == len(pair) - 1)
                            nc.tensor.matmul(
                                psum_o[:, c0:c1], v_sb[:, kt, :],
                                e2[:, j, c0:c1],
                                start=first, stop=last)
                    # free psum_o with a plain copy (Pool; ACT/DVE are
                    # busy) and normalize in SBUF once the reciprocal-
                    # broadcast chain lands; the final group normalizes
                    # straight from PSUM (nothing follows it)
                    den_sb = sbB.tile([1, QB], F32, tag="densb")
                    nc.scalar.activation(
                        den_sb[:], psum_den[:],
                        mybir.ActivationFunctionType.Identity,
                        bias=exps_sb[0:1, h:h + 1])
                    r_sb = sbB.tile([1, QB], F32, tag="rsb")
                    nc.vector.reciprocal(r_sb[:], den_sb[:])
                    rb = sbB.tile([128, QB], F32, tag="rb")
                    nc.gpsimd.partition_broadcast(rb[:], r_sb[:])
                    if last_group:
                        nc.vector.tensor_mul(oT[h][:, qs], psum_o[:], rb[:])
                    else:
                        o_un = sbB.tile([128, QB], F32, tag="oun")
                        nc.vector.tensor_copy(o_un[:], psum_o[:])
                        nc.vector.tensor_mul(oT[h][:, qs], o_un[:], rb[:])

                # ---- Wo for this q block ----
                for j in range(QB // 128):
                    qt = qb * (QB // 128) + j
                    qts = slice(qt * 128, (qt + 1) * 128)
                    y_slab = sbY.tile([128, D], BF16, tag="ysb")
                    for nb in range(D // QB):
                        ns = slice(nb * QB, (nb + 1) * QB)
                        psum_y = psY.tile([128, QB], F32, tag="y")
                        for h in range(2):
                            nc.tensor.matmul(
                                psum_y[:],
                                oT[h][:, qts],
                                wo_sb[:, h, ns],
                                start=(h == 0), stop=(h == 1))
                        if (qt + nb) % 2 == 0:
                            nc.vector.tensor_copy(y_slab[:, ns], psum_y[:])
                        else:
                            nc.gpsimd.tensor_copy(y_slab[:, ns], psum_y[:])
                        if nb == 1 or nb == 3:
                            nc.sync.dma_start(
                                y_d[qts, (nb - 1) * QB:(nb + 1) * QB],
                                y_slab[:, (nb - 1) * QB:(nb + 1) * QB])

    nc.compile()
    return nc


# head-dim permutation: rotation pair (i, i+64) -> partitions (2i, 2i+1);
# non-rotating dims 32:64 -> 64:96, 96:128 stay.
PERM = np.zeros(HD, dtype=np.int64)
for _i in range(32):
    PERM[2 * _i] = _i
    PERM[2 * _i + 1] = 64 + _i
for _j in range(32):
    PERM[64 + _j] = 32 + _j
    PERM[96 + _j] = 96 + _j


def _rope_tables():
    import ml_dtypes
    freqs = (1.0 / ROPE_BASE) ** np.linspace(0.0, 1.0, num=HD // 4,
                                             dtype=np.float32)
    theta = freqs[:, None].astype(np.float32) * np.arange(L, dtype=np.float32)[None, :]
    cos32 = np.cos(theta).astype(np.float32)
    sin32 = np.sin(theta).astype(np.float32)
    cos64 = np.empty((64, L), dtype=np.float32)
    sin64 = np.empty((64, L), dtype=np.float32)
    cos64[0::2] = cos32
    cos64[1::2] = cos32
    sin64[0::2] = sin32
    sin64[1::2] = -sin32
    return (cos64.astype(ml_dtypes.bfloat16), sin64.astype(ml_dtypes.bfloat16))


def _make_in_maps(x, Wqkv, Wo, s):
    import ml_dtypes
    bf16 = ml_dtypes.bfloat16
    x = np.asarray(x, dtype=np.float32)
    Wqkv = np.asarray(Wqkv, dtype=np.float32)
    Wo = np.asarray(Wo, dtype=np.float32)
    s = np.asarray(s, dtype=np.float32)
    # swizzle into SBUF layout [partition, chunk, l]:
    # xT_sw[r, k, l] = x[l, 128k + r]
    xT = np.ascontiguousarray(
        x.reshape(L, NDK, 128).transpose(2, 1, 0)).astype(bf16)
    cosd, sind = _rope_tables()
    in_maps = []
    for c in range(N_CORES):
        g = c // 2
        qcols = [Wqkv[:, (2 * c + hh) * HD:(2 * c + hh + 1) * HD][:, PERM]
                 for hh in range(2)]
        kcols = Wqkv[:, 16 * HD + g * HD:16 * HD + (g + 1) * HD][:, PERM]
        vcols = Wqkv[:, 20 * HD + g * HD:20 * HD + (g + 1) * HD]
        wslc = np.concatenate(qcols + [kcols, vcols], axis=1)
        # wslc_sw[r, k, cc] = wslc[128k + r, cc]
        wslc_sw = np.ascontiguousarray(
            wslc.reshape(NDK, 128, 4 * HD).transpose(1, 0, 2)).astype(bf16)
        in_maps.append({
            "xT": xT,
            "wslc": wslc_sw,
            "wo": np.ascontiguousarray(Wo[(2 * c) * HD:(2 * c + 2) * HD, :]).astype(bf16),
            "snk": np.ascontiguousarray(s[:, 2 * c:2 * c + 2]),
            "cosd": cosd,
            "sind": sind,
        })
    return in_maps


_CACHE = {}


def _get_exec():
    """Build the program once and return a cached jitted 8-core executor."""
    if "exec" in _CACHE:
        return _CACHE["exec"]

    import jax
    from jax.sharding import Mesh, PartitionSpec
    from jax.experimental.shard_map import shard_map
    from concourse.bass2jax import (_bass_exec_p, install_neuronx_cc_hook,
                                    partition_id_tensor)

    nc = _build_program()
    _CACHE["nc"] = nc
    install_neuronx_cc_hook()

    partition_name = (nc.partition_id_tensor.name
                      if nc.partition_id_tensor else None)
    in_names, out_names, out_avals = [], [], []
    for alloc in nc.m.functions[0].allocations:
        if not isinstance(alloc, mybir.MemoryLocationSet):
            continue
        name = alloc.memorylocations[0].name
        if alloc.kind == "ExternalInput":
            if name != partition_name:
                in_names.append(name)
        elif alloc.kind == "ExternalOutput":
            out_names.append(name)
            out_avals.append(jax.core.ShapedArray(
                tuple(alloc.tensor_shape), mybir.dt.np(alloc.dtype)))
    n_params = len(in_names)
    all_names = in_names + out_names
    if partition_name is not None:
        all_names = all_names + [partition_name]

    def _body(*args):
        operands = list(args)
        if partition_name is not None:
            operands.append(partition_id_tensor())
        outs = _bass_exec_p.bind(
            *operands,
            out_avals=tuple(out_avals),
            in_names=tuple(all_names),
            out_names=tuple(out_names),
            lowering_input_output_aliases=(),
            sim_require_finite=True,
            sim_require_nnan=True,
            nc=nc,
        )
        return tuple(outs)

    devices = jax.devices()[:N_CORES]
    mesh = Mesh(np.asarray(devices), ("core",))
    n_outs = len(out_names)
    sharded = jax.jit(
        shard_map(_body, mesh=mesh,
                  in_specs=(PartitionSpec("core"),) * (n_params + n_outs),
                  out_specs=(PartitionSpec("core"),) * n_outs,
                  check_rep=False),
        keep_unused=True)

    state = {
        "sharded": sharded, "in_names": in_names, "out_names": out_names,
        "out_avals": out_avals, "mesh": mesh, "n_params": n_params,
    }
    _CACHE["exec"] = state
    return state


def _run_cores(in_maps):
    ex = _get_exec()
    concat_in = [
        np.concatenate([np.asarray(m[name]) for m in in_maps], axis=0)
        for name in ex["in_names"]
    ]
    concat_zeros = [
        np.zeros((N_CORES * a.shape[0],) + tuple(a.shape[1:]), a.dtype)
        for a in ex["out_avals"]
    ]
    outs = ex["sharded"](*concat_in, *concat_zeros)
    name_to_i = {n: i for i, n in enumerate(ex["out_names"])}
    yi = name_to_i["y"]
    y_all = np.asarray(outs[yi]).reshape(N_CORES, L, D)
    return y_all


def kernel(x, Wqkv, Wo, s):
    in_maps = _make_in_maps(x, Wqkv, Wo, s)
    y_all = _run_cores(in_maps)
    out = y_all.astype(np.float32).sum(axis=0)
    return out.reshape(1, L, D).astype(np.float32)


# revision 43
# speedup vs baseline: 1.4136x; 1.0822x over previous
"""Trainium2 Bass kernel for GroupedQueryAttention (inverted sliding-window mask + sink).

Full inputs in, full output out. Internally head-sharded across 8 NeuronCores:
core c handles q heads {2c, 2c+1} and kv head c//2, computes its partial
(x @ Wqkv_slice -> RoPE -> scores -> masked softmax w/ sink -> AV -> @ Wo_slice),
host sums the 8 partial outputs (the all-reduce).

v3 design notes:
- bf16 datapath end to end (inputs, SBUF slabs, output partials).
- head-dim rows of q/k are permuted on the host so each RoPE rotation pair
  sits on adjacent partitions; the partner operand is then a single DVE
  stream_shuffle (swap-adjacent within quadrants) -- no cross-partition DMA.
  Rows 64:128 of the permuted layout don't rotate and are left untouched.
- xT is preloaded whole into SBUF; projection runs in two l-block pairs
  consuming xT chunks in DMA arrival order.
- inverted-band mask: score/exp/AV/den matmuls are column-trimmed to the
  active range of each (k-tile, q-block) tile; the one 128-col boundary
  stripe is zeroed with a multiplicative bf16 mask on DVE.
- y partials are written as [128, 2048] slabs (16 output DMAs).
"""

import os
import sys
from contextlib import ExitStack

sys.path.insert(0, "/opt/trn_rl_repo")

# jax must see the axon/neuron platform; a stray JAX_PLATFORMS=cpu would hide it.
if os.environ.get("JAX_PLATFORMS", "") == "cpu":
    os.environ["JAX_PLATFORMS"] = ""

import numpy as np

import concourse.bass as bass
import concourse.tile as tile
from concourse import bacc, mybir

F32 = mybir.dt.float32
BF16 = mybir.dt.bfloat16

N_CORES = 8
L = 2048
D = 2048
HD = 128
WINDOW = 1024
ROPE_BASE = 1024.0
SM_SCALE = 1.0 / float(np.sqrt(HD))

MASK_VAL = -1.0e6
QB = 512          # q block (free dim of score tiles)
NQB = L // QB     # 4
NKT = L // HD     # 16 k tiles of 128
NDK = D // HD     # 16 contraction chunks for projections
NLB = L // QB     # 4 l-blocks for projection

LOWER_D0S = (0, -128, -256, -384)
UPPER_D0S = (640, 768, 896, 1024)
MASK_IDX = {d: i for i, d in enumerate(LOWER_D0S + UPPER_D0S)}

# swap-adjacent stream_shuffle mask (within each 32-partition quadrant)
SWAP_ADJ = [j ^ 1 for j in range(32)]

# qb=1's first tiles only need l-blocks 1-2, hiding the tail of phase A
QB_ORDER = (1, 2, 3, 0)
# projection l-block order: lb0 last, so attention on qb1 (which needs only
# lb1-3) starts the moment projection ends; lb0's v-transposes spill into
# phase B
LB_ORDER = (1, 2, 3, 0)
LB0 = LB_ORDER[0]


def _classify(kt: int, qb: int):
    """masked band is 0 <= q-k <= WINDOW-1 (those entries are dropped).

    Returns (kind, d0, c0, c1, m0, m1): active column range [c0, c1) and
    boundary mask-stripe [m0, m1) for the (k-tile, q-block) tile.
    """
    d0 = QB * qb - HD * kt
    if 128 <= d0 <= 512:
        return "skip", d0, 0, 0, 0, 0
    if d0 <= -512 or d0 >= 1152:
        return "full", d0, 0, QB, 0, 0
    if d0 <= 0:
        w = min(QB, 128 - d0)
        return "lower", d0, 0, w, w - 128, w
    off = max(0, 1024 - d0)
    return "upper", d0, off, QB, off, off + 128


def _build_program():
    nc = bacc.Bacc("TRN2", target_bir_lowering=False, debug=False,
                   num_devices=N_CORES)

    # xT and wslc are pre-swizzled on the host into SBUF layout
    # ([partition, chunk, free]) so input DMAs are few and pattern-identical.
    xT_d = nc.dram_tensor("xT", [128, NDK, L], BF16, kind="ExternalInput").ap()
    wslc_d = nc.dram_tensor("wslc", [128, NDK, 4 * HD], BF16,
                            kind="ExternalInput").ap()
    wo_d = nc.dram_tensor("wo", [2 * HD, D], BF16, kind="ExternalInput").ap()
    snk_d = nc.dram_tensor("snk", [1, 2], F32, kind="ExternalInput").ap()
    cosd_d = nc.dram_tensor("cosd", [64, L], BF16, kind="ExternalInput").ap()
    sind_d = nc.dram_tensor("sind", [64, L], BF16, kind="ExternalInput").ap()
    boot_d = nc.dram_tensor("boot", [128, 4, 512], BF16,
                            kind="ExternalInput").ap()
    y_d = nc.dram_tensor("y", [L, D], BF16, kind="ExternalOutput").ap()

    with tile.TileContext(nc) as tc, ExitStack() as stk:
        persist = stk.enter_context(tc.tile_pool(name="persist", bufs=1))

        # ---- persistent SBUF tensors ----
        xT_sb = persist.tile([128, NDK, L], BF16, tag="xT")
        wslc_sb = persist.tile([128, NDK, 4 * HD], BF16, tag="wslc")
        wo_sb = persist.tile([128, 2, D], BF16, tag="wo")
        qT = [persist.tile([128, L], BF16, tag=f"qT{h}", name=f"qT{h}") for h in range(2)]
        kT = persist.tile([128, L], BF16, tag="kT")
        v_sb = persist.tile([128, NKT, HD], BF16, tag="v")
        oT = [persist.tile([128, L], BF16, tag=f"oT{h}", name=f"oT{h}") for h in range(2)]
        cosd_sb = persist.tile([64, L], BF16, tag="cosd")
        sind_sb = persist.tile([64, L], BF16, tag="sind")
        # additive boundary stripes: stripes[:,0,:] masks where c >= k'
        # (lower-edge tiles), stripes[:,1,:] masks where c < k' (upper-edge)
        stripes = persist.tile([128, 2, 128], BF16, tag="stripes")
        ident = persist.tile([128, 128], BF16, tag="ident")
        ones_f32 = persist.tile([128, 1], F32, tag="onesf")
        ones_sb = persist.tile([128, 1], BF16, tag="ones")
        boot_sb = persist.tile([128, 4, 512], BF16, tag="boot")
        snk_sb = persist.tile([1, 2], F32, tag="snk")
        exps_sb = persist.tile([1, 2], F32, tag="exps")

        # ---- input DMAs ----
        # sync queue: wslc + xT interleaved in consumption order.  lb0's
        # x-tiles arrive in fine k-group pieces so the first projection
        # matmuls start ~2.5us in; later l-blocks use coarser pieces.
        lb0 = LB_ORDER[0]
        fs = slice(lb0 * QB, (lb0 + 1) * QB)
        # boot: wslc chunks 0-1 + first l-block's x chunks 0-1 in one DMA
        nc.sync.dma_start(boot_sb[:], boot_d[:])
        nc.sync.dma_start(wslc_sb[:, 0:2, :], wslc_d[:, 0:2, :])
        nc.sync.dma_start(wslc_sb[:, 2:4, :], wslc_d[:, 2:4, :])
        nc.sync.dma_start(xT_sb[:, 2:4, fs], xT_d[:, 2:4, fs])
        nc.sync.dma_start(wslc_sb[:, 4:6, :], wslc_d[:, 4:6, :])
        nc.sync.dma_start(xT_sb[:, 4:6, fs], xT_d[:, 4:6, fs])
        nc.sync.dma_start(wslc_sb[:, 6:8, :], wslc_d[:, 6:8, :])
        nc.sync.dma_start(xT_sb[:, 6:8, fs], xT_d[:, 6:8, fs])
        nc.sync.dma_start(wslc_sb[:, 8:12, :], wslc_d[:, 8:12, :])
        nc.sync.dma_start(xT_sb[:, 8:12, fs], xT_d[:, 8:12, fs])
        nc.sync.dma_start(wslc_sb[:, 12:16, :], wslc_d[:, 12:16, :])
        nc.sync.dma_start(xT_sb[:, 12:16, fs], xT_d[:, 12:16, fs])
        for lb in LB_ORDER[1:]:
            ls = slice(lb * QB, (lb + 1) * QB)
            nc.sync.dma_start(xT_sb[:, 0:8, ls], xT_d[:, 0:8, ls])
            nc.sync.dma_start(xT_sb[:, 8:16, ls], xT_d[:, 8:16, ls])
        # gpsimd (software-DGE) queue: rope tables, sink, Wo
        nc.gpsimd.dma_start(cosd_sb[:], cosd_d[:])
        nc.gpsimd.dma_start(sind_sb[:], sind_d[:])
        nc.gpsimd.dma_start(snk_sb[:], snk_d[:])
        for h in range(2):
            nc.gpsimd.dma_start(wo_sb[:, h, :], wo_d[h * 128:(h + 1) * 128, :])

        # ---- constants ----
        nc.gpsimd.memset(ones_f32[:], 1.0)
        nc.scalar.copy(ones_sb[:], ones_f32[:])
        # identity for PE transposes
        nc.gpsimd.memset(ident[:], 0.0)
        nc.gpsimd.affine_select(
            out=ident[:], in_=ident[:], compare_op=mybir.AluOpType.not_equal,
            fill=1.0, base=0, channel_multiplier=1, pattern=[[-1, 128]])
        # additive boundary stripes (0 kept, -1e6 dropped); every partial
        # tile's masked wedge is one of these two patterns at offset m0
        for i in range(2):
            m = stripes[:, i, :]
            nc.gpsimd.memset(m, 0.0)
            if i == 0:
                # lower edge: keep where c < k'
                nc.gpsimd.affine_select(
                    out=m, in_=m, compare_op=mybir.AluOpType.is_ge,
                    fill=MASK_VAL, base=-1, channel_multiplier=1,
                    pattern=[[-1, 128]])
            else:
                # upper edge: keep where c >= k'
                nc.gpsimd.affine_select(
                    out=m, in_=m, compare_op=mybir.AluOpType.is_ge,
                    fill=MASK_VAL, base=0, channel_multiplier=-1,
                    pattern=[[1, 128]])
        # exp of the two sink logits
        nc.scalar.activation(exps_sb[:], snk_sb[:], mybir.ActivationFunctionType.Exp)

        # ================= Phase A: QKV projection (transposed) =================
        # pT[c*128+r, l] = sum_d wslc[d, c*128+r] * x[l, d];  cols c = q0,q1,k,v
        vt_pool = stk.enter_context(tc.tile_pool(name="vtp", bufs=2))
        rope_pool = stk.enter_context(tc.tile_pool(name="rope", bufs=2))
        sbB = stk.enter_context(tc.tile_pool(name="sbB", bufs=4))
        sbE = stk.enter_context(tc.tile_pool(name="sbE", bufs=8))
        sbY = stk.enter_context(tc.tile_pool(name="sbY", bufs=3))
        with tc.tile_pool(name="psA", bufs=8, space="PSUM") as psA:
            vt_q = []

            def _emit_v_transposes(item):
                dlb, dvt = item
                for j in range(QB // 128):
                    kt = dlb * (QB // 128) + j
                    pt = psA.tile([128, 128], BF16, tag="proj")
                    nc.tensor.transpose(pt[:], dvt[:, j * 128:(j + 1) * 128],
                                        ident[:])
                    nc.scalar.copy(v_sb[:, kt, :], pt[:])

            for lb in LB_ORDER:
                ls = slice(lb * QB, (lb + 1) * QB)
                ps = [psA.tile([128, QB], F32, tag="proj", name=f"ps{lb}c{c}")
                      for c in range(4)]
                first_lb = (lb == LB_ORDER[0])
                for k in range(NDK):
                    if k == 5 and vt_q:
                        _emit_v_transposes(vt_q.pop(0))
                    if first_lb and k < 2:
                        w_ap = boot_sb[:, 2 * k, :]
                        x_ap = boot_sb[:, 2 * k + 1, :]
                    else:
                        w_ap = wslc_sb[:, k, :]
                        x_ap = xT_sb[:, k, ls]
                    for c in range(4):
                        nc.tensor.matmul(
                            ps[c][:],
                            w_ap[:, c * 128:(c + 1) * 128] if True else None,
                            x_ap,
                            start=(k == 0), stop=(k == NDK - 1))
                # drain psums: q0,q1 on ACT; k,v on DVE
                nc.scalar.copy(qT[0][:, ls], ps[0][:])
                nc.scalar.copy(qT[1][:, ls], ps[1][:])
                nc.vector.tensor_copy(kT[:, ls], ps[2][:])
                vT_lb = vt_pool.tile([128, QB], BF16, tag="vt")
                nc.vector.tensor_copy(vT_lb[:], ps[3][:])

                # ---- RoPE on rows 0:64 of this l-block (in place) ----
                for t in (qT[0], qT[1], kT):
                    u = rope_pool.tile([64, QB], BF16, tag="u")
                    tmp = rope_pool.tile([64, QB], BF16, tag="tmp")
                    nc.vector.stream_shuffle(u[:], t[0:64, ls], SWAP_ADJ)
                    nc.vector.tensor_mul(tmp[:], t[0:64, ls], cosd_sb[:, ls])
                    nc.vector.tensor_mul(u[:], u[:], sind_sb[:, ls])
                    nc.vector.tensor_add(t[0:64, ls], tmp[:], u[:])

                # ---- v: transpose to natural (k, d) tiles, deferred one
                # l-block so the PE never waits on the vT drain copy ----
                vt_q.append((lb, vT_lb))

        # ============ Phase B+C: attention + output projection ============
        # Full tiles are processed in pairs sharing a two-bank PSUM tile so
        # one ACT exp instruction covers both (halves ACT instruction count);
        # partial (boundary) tiles follow, column-trimmed to the active range.
        with tc.tile_pool(name="psS", bufs=2, space="PSUM") as psS, \
             tc.tile_pool(name="psO", bufs=1, space="PSUM") as psO, \
             tc.tile_pool(name="psD", bufs=1, space="PSUM") as psD, \
             tc.tile_pool(name="psY", bufs=2, space="PSUM") as psY:
            for qb in QB_ORDER:
                qs = slice(qb * QB, (qb + 1) * QB)
                for h in range(2):
                    acts = [(kt, *_classify(kt, qb)) for kt in range(NKT)]
                    fulls = [a for a in acts if a[1] == "full"]
                    parts = [a for a in acts if a[1] in ("lower", "upper")]
                    n_act = len(fulls) + len(parts)
                    # pair partials with equal active ranges when possible
                    from collections import defaultdict as _dd
                    byrange = _dd(list)
                    for a in parts:
                        byrange[(a[3], a[4])].append(a)
                    ppairs, odd = [], []
                    for key in sorted(byrange):
                        grp = byrange[key]
                        while len(grp) >= 2:
                            ppairs.append([grp.pop(0), grp.pop(0)])
                        if grp:
                            odd.append(grp.pop())
                    while len(odd) >= 2:
                        ppairs.append([odd.pop(0), odd.pop(0)])
                    if odd:
                        ppairs.append([odd.pop()])
                    fpairs = [fulls[p:p + 2] for p in range(0, len(fulls), 2)]
                    # full pairs first (the first opens all 512 psum cols),
                    # then the column-trimmed partial pairs
                    order = fpairs + ppairs
                    psum_o = psO.tile([128, QB], F32, tag="o")
                    psum_den = psD.tile([1, QB], F32, tag="den")
                    i = 0
                    last_group = (qb == QB_ORDER[-1] and h == 1)
                    deferred_av = []
                    # den instruction count: one per equal-range pair, one
                    # per half otherwise
                    # den instruction count: full pairs fold 2:1 via esum,
                    # and consecutive full pairs fold once more (quad); the
                    # FIRST full pair keeps its own den (it must open the
                    # [0:512] accumulation), quads start from pair index 1.
                    nfp = len(fpairs)
                    quad_peers = {}
                    p_idx = 1
                    while p_idx + 1 < nfp:
                        quad_peers[p_idx] = "hold"
                        quad_peers[p_idx + 1] = "fold"
                        p_idx += 2
                    n_den = 1 + sum(1 for v in quad_peers.values()
                                    if v == "fold") + \
                        (1 if (nfp - 1) % 2 == 1 else 0) + \
                        sum(1 if (len(p) == 2 and (p[0][3], p[0][4]) ==
                                  (p[1][3], p[1][4])) else len(p)
                            for p in ppairs)
                    den_i = 0
                    pend_esum = None

                    def av_den(kt, e_ap, cs, first, last):
                        nc.tensor.matmul(
                            psum_den[:, cs], ones_sb[:], e_ap,
                            start=first, stop=last)
                        nc.tensor.matmul(
                            psum_o[:, cs], v_sb[:, kt, :], e_ap,
                            start=first, stop=last)

                    for pi, pair in enumerate(order):
                        if pi == 1:
                            while vt_q:
                                dlb, dvt = vt_q.pop(0)
                                for jv in range(QB // 128):
                                    ktv = dlb * (QB // 128) + jv
                                    ptv = psY.tile([128, 128], BF16, tag="y",
                                                   name=f"vt{ktv}")
                                    nc.tensor.transpose(
                                        ptv[:], dvt[:, jv * 128:(jv + 1) * 128],
                                        ident[:])
                                    nc.scalar.copy(v_sb[:, ktv, :], ptv[:])
                        ps2 = psS.tile([128, 2, QB], F32, tag="s")
                        e2 = sbE.tile([128, 2, QB], BF16, tag="e")
                        for j, (kt, cls, d0, c0, c1, m0, m1) in enumerate(pair):
                            cs = slice(c0, c1)
                            nc.tensor.matmul(
                                ps2[:, j, cs],
                                kT[:, kt * 128:(kt + 1) * 128],
                                qT[h][:, qb * QB + c0:qb * QB + c1],
                                start=True, stop=(cls == "full"))
                            if cls != "full":
                                # add the -1e6 boundary stripe in PSUM (PE,
                                # N=128) so exp sees pre-masked scores
                                nc.tensor.matmul(
                                    ps2[:, j, m0:m1], ident[:],
                                    stripes[:, 0 if cls == "lower" else 1, :],
                                    start=False, stop=True)
                        r0 = (pair[0][3], pair[0][4])
                        r1 = (pair[-1][3], pair[-1][4])
                        if r0 == r1 and len(pair) == 2:
                            # same active range: one exp over both halves
                            nc.scalar.activation(
                                e2[:, :, r0[0]:r0[1]], ps2[:, :, r0[0]:r0[1]],
                                mybir.ActivationFunctionType.Exp,
                                scale=SM_SCALE)
                        else:
                            for j, (kt, cls, d0, c0, c1, m0, m1) in enumerate(pair):
                                nc.scalar.activation(
                                    e2[:, j, c0:c1], ps2[:, j, c0:c1],
                                    mybir.ActivationFunctionType.Exp,
                                    scale=SM_SCALE)
                        eqrange = (len(pair) == 2 and
                                   (pair[0][3], pair[0][4]) ==
                                   (pair[1][3], pair[1][4]))
                        if eqrange:
                            # fold the pair's den into one matmul via a bf16
                            # DVE add of the two e halves; consecutive full
                            # pairs fold once more (4 tiles -> 1 den matmul)
                            c0, c1 = pair[0][3], pair[0][4]
                            esum = sbB.tile([128, QB], BF16, tag="esum")
                            nc.vector.tensor_add(
                                esum[:, c0:c1], e2[:, 0, c0:c1],
                                e2[:, 1, c0:c1])
                            role = quad_peers.get(pi)
                            if role == "hold":
                                pend_esum = esum
                            else:
                                if role == "fold":
                                    nc.vector.tensor_add(
                                        esum[:], esum[:], pend_esum[:])
                                    pend_esum = None
                                nc.tensor.matmul(
                                    psum_den[:, c0:c1], ones_sb[:],
                                    esum[:, c0:c1],
                                    start=(den_i == 0),
                                    stop=(den_i == n_den - 1))
                                den_i += 1
                        else:
                            for j, (kt, cls, d0, c0, c1, m0, m1) in enumerate(pair):
                                nc.tensor.matmul(
                                    psum_den[:, c0:c1], ones_sb[:],
                                    e2[:, j, c0:c1],
                                    start=(den_i == 0), stop=(den_i == n_den - 1))
                                den_i += 1
                        if last_group:
                            deferred_av.append((pair, e2))
                            i += len(pair)
                        else:
                            for j, (kt, cls, d0, c0, c1, m0, m1) in enumerate(pair):
                                nc.tensor.matmul(
                                    psum_o[:, c0:c1], v_sb[:, kt, :],
                                    e2[:, j, c0:c1],
                                    start=(i == 0), stop=(i == n_act - 1))
                                i += 1
                    for idx, (pair, e2) in enumerate(deferred_av):
                        for j, (kt, cls, d0, c0, c1, m0, m1) in enumerate(pair):
                            first = (idx == 0 and j == 0)
                            last = (idx == len(deferred_av) - 1
                                    and j == len(pair) - 1)
                            nc.tensor.matmul(
                                psum_o[:, c0:c1], v_sb[:, kt, :],
                                e2[:, j, c0:c1],
                                start=first, stop=last)
                    # free psum_o with a plain copy (Pool; ACT/DVE are
                    # busy) and normalize in SBUF once the reciprocal-
                    # broadcast chain lands; the final group normalizes
                    # straight from PSUM (nothing follows it)
                    den_sb = sbB.tile([1, QB], F32, tag="densb")
                    nc.vector.tensor_scalar_add(
                        den_sb[:], psum_den[:], exps_sb[0:1, h:h + 1])
                    r_sb = sbB.tile([1, QB], F32, tag="rsb")
                    nc.vector.reciprocal(r_sb[:], den_sb[:])
                    rb = sbB.tile([128, QB], F32, tag="rb")
                    nc.gpsimd.partition_broadcast(rb[:], r_sb[:])
                    if last_group:
                        nc.vector.tensor_mul(oT[h][:, qs], psum_o[:], rb[:])
                    else:
                        o_un = sbB.tile([128, QB], F32, tag="oun")
                        nc.vector.tensor_copy(o_un[:], psum_o[:])
                        nc.vector.tensor_mul(oT[h][:, qs], o_un[:], rb[:])

                # ---- Wo for this q block ----
                lastqb = (qb == QB_ORDER[-1])
                for j in range(QB // 128):
                    qt = qb * (QB // 128) + j
                    qts = slice(qt * 128, (qt + 1) * 128)
                    y_slab = sbY.tile([128, D], BF16, tag="ysb")
                    for nb in range(D // QB):
                        ns = slice(nb * QB, (nb + 1) * QB)
                        if lastqb:
                            # attention is done: borrow the idle psS banks
                            if nb % 2 == 0:
                                py2 = psS.tile([128, 2, QB], F32, tag="s",
                                               name=f"wo{qt}n{nb}")
                            psum_y = py2[:, nb % 2, :]
                        else:
                            psum_y = psY.tile([128, QB], F32, tag="y")[:]
                        for h in range(2):
                            nc.tensor.matmul(
                                psum_y,
                                oT[h][:, qts],
                                wo_sb[:, h, ns],
                                start=(h == 0), stop=(h == 1))
                        if lastqb and (qt + nb) % 2 == 1:
                            nc.scalar.copy(y_slab[:, ns], psum_y)
                        else:
                            nc.vector.tensor_copy(y_slab[:, ns], psum_y)
                        if lastqb and qt >= 14:
                            nc.sync.dma_start(y_d[qts, ns], y_slab[:, ns])
                        elif nb == 1 or nb == 3:
                            nc.sync.dma_start(
                                y_d[qts, (nb - 1) * QB:(nb + 1) * QB],
                                y_slab[:, (nb - 1) * QB:(nb + 1) * QB])

    nc.compile()
    return nc


# head-dim permutation: rotation pair (i, i+64) -> partitions (2i, 2i+1);
# non-rotating dims 32:64 -> 64:96, 96:128 stay.
PERM = np.zeros(HD, dtype=np.int64)
for _i in range(32):
    PERM[2 * _i] = _i
    PERM[2 * _i + 1] = 64 + _i
for _j in range(32):
    PERM[64 + _j] = 32 + _j
    PERM[96 + _j] = 96 + _j


def _rope_tables():
    import ml_dtypes
    freqs = (1.0 / ROPE_BASE) ** np.linspace(0.0, 1.0, num=HD // 4,
                                             dtype=np.float32)
    theta = freqs[:, None].astype(np.float32) * np.arange(L, dtype=np.float32)[None, :]
    cos32 = np.cos(theta).astype(np.float32)
    sin32 = np.sin(theta).astype(np.float32)
    cos64 = np.empty((64, L), dtype=np.float32)
    sin64 = np.empty((64, L), dtype=np.float32)
    cos64[0::2] = cos32
    cos64[1::2] = cos32
    sin64[0::2] = sin32
    sin64[1::2] = -sin32
    return (cos64.astype(ml_dtypes.bfloat16), sin64.astype(ml_dtypes.bfloat16))


def _make_in_maps(x, Wqkv, Wo, s):
    import ml_dtypes
    bf16 = ml_dtypes.bfloat16
    x = np.asarray(x, dtype=np.float32)
    Wqkv = np.asarray(Wqkv, dtype=np.float32)
    Wo = np.asarray(Wo, dtype=np.float32)
    s = np.asarray(s, dtype=np.float32)
    # swizzle into SBUF layout [partition, chunk, l]:
    # xT_sw[r, k, l] = x[l, 128k + r]
    xT = np.ascontiguousarray(
        x.reshape(L, NDK, 128).transpose(2, 1, 0)).astype(bf16)
    cosd, sind = _rope_tables()
    in_maps = []
    for c in range(N_CORES):
        g = c // 2
        qcols = [Wqkv[:, (2 * c + hh) * HD:(2 * c + hh + 1) * HD][:, PERM]
                 for hh in range(2)]
        kcols = Wqkv[:, 16 * HD + g * HD:16 * HD + (g + 1) * HD][:, PERM]
        vcols = Wqkv[:, 20 * HD + g * HD:20 * HD + (g + 1) * HD]
        wslc = np.concatenate(qcols + [kcols, vcols], axis=1)
        # wslc_sw[r, k, cc] = wslc[128k + r, cc]
        wslc_sw = np.ascontiguousarray(
            wslc.reshape(NDK, 128, 4 * HD).transpose(1, 0, 2)).astype(bf16)
        boot = np.concatenate([
            wslc_sw[:, 0, :][:, None, :],
            xT[:, 0, LB0 * QB:(LB0 + 1) * QB][:, None, :],
            wslc_sw[:, 1, :][:, None, :],
            xT[:, 1, LB0 * QB:(LB0 + 1) * QB][:, None, :],
        ], axis=1)
        in_maps.append({
            "xT": xT,
            "wslc": wslc_sw,
            "boot": np.ascontiguousarray(boot),
            "wo": np.ascontiguousarray(Wo[(2 * c) * HD:(2 * c + 2) * HD, :]).astype(bf16),
            "snk": np.ascontiguousarray(s[:, 2 * c:2 * c + 2]),
            "cosd": cosd,
            "sind": sind,
        })
    return in_maps


_CACHE = {}


def _get_exec():
    """Build the program once and return a cached jitted 8-core executor."""
    if "exec" in _CACHE:
        return _CACHE["exec"]

    import jax
    from jax.sharding import Mesh, PartitionSpec
    from jax.experimental.shard_map import shard_map
    from concourse.bass2jax import (_bass_exec_p, install_neuronx_cc_hook,
                                    partition_id_tensor)

    nc = _build_program()
    _CACHE["nc"] = nc
    install_neuronx_cc_hook()

    partition_name = (nc.partition_id_tensor.name
                      if nc.partition_id_tensor else None)
    in_names, out_names, out_avals = [], [], []
    for alloc in nc.m.functions[0].allocations:
        if not isinstance(alloc, mybir.MemoryLocationSet):
            continue
        name = alloc.memorylocations[0].name
        if alloc.kind == "ExternalInput":
            if name != partition_name:
                in_names.append(name)
        elif alloc.kind == "ExternalOutput":
            out_names.append(name)
            out_avals.append(jax.core.ShapedArray(
                tuple(alloc.tensor_shape), mybir.dt.np(alloc.dtype)))
    n_params = len(in_names)
    all_names = in_names + out_names
    if partition_name is not None:
        all_names = all_names + [partition_name]

    def _body(*args):
        operands = list(args)
        if partition_name is not None:
            operands.append(partition_id_tensor())
        outs = _bass_exec_p.bind(
            *operands,
            out_avals=tuple(out_avals),
            in_names=tuple(all_names),
            out_names=tuple(out_names),
            lowering_input_output_aliases=(),
            sim_require_finite=True,
            sim_require_nnan=True,
            nc=nc,
        )
        return tuple(outs)

    devices = jax.devices()[:N_CORES]
    mesh = Mesh(np.asarray(devices), ("core",))
    n_outs = len(out_names)
    sharded = jax.jit(
        shard_map(_body, mesh=mesh,
                  in_specs=(PartitionSpec("core"),) * (n_params + n_outs),
                  out_specs=(PartitionSpec("core"),) * n_outs,
                  check_rep=False),
        keep_unused=True)

    state = {
        "sharded": sharded, "in_names": in_names, "out_names": out_names,
        "out_avals": out_avals, "mesh": mesh, "n_params": n_params,
    }
    _CACHE["exec"] = state
    return state


def _run_cores(in_maps):
    ex = _get_exec()
    concat_in = [
        np.concatenate([np.asarray(m[name]) for m in in_maps], axis=0)
        for name in ex["in_names"]
    ]
    concat_zeros = [
        np.zeros((N_CORES * a.shape[0],) + tuple(a.shape[1:]), a.dtype)
        for a in ex["out_avals"]
    ]
    outs = ex["sharded"](*concat_in, *concat_zeros)
    name_to_i = {n: i for i, n in enumerate(ex["out_names"])}
    yi = name_to_i["y"]
    y_all = np.asarray(outs[yi]).reshape(N_CORES, L, D)
    return y_all


def kernel(x, Wqkv, Wo, s):
    in_maps = _make_in_maps(x, Wqkv, Wo, s)
    y_all = _run_cores(in_maps)
    out = y_all.astype(np.float32).sum(axis=0)
    return out.reshape(1, L, D).astype(np.float32)
